# revision 7
# baseline (speedup 1.0000x reference)
"""Trainium2 Bass kernel for nn_CotLayer (CoT attention layer).

Computation (see reference):
  kemb = relu(grouped_conv3x3(x, Wk, groups=4))
  w1   = relu(We1 @ [x; kemb])            (1x1)
  wbar_k = We2_k @ w1 + be2_k             (per-pixel 3x3 kernel, 8-fold
                                           group replication folded into We2_k)
  xv   = Wv @ x                           (1x1)
  agg  = relu(sum_k shift_k(xv) * wbar_k)
  gap  = mean_{H,W}(agg + kemb)           (AllReduce across 4-core groups)
  attn = sigmoid pair of SE-MLP(gap)
  out  = agg*attn0 + kemb*attn1

Sharding: 8 cores = (batch b) x (H-quarter q); each core computes 64 output
rows; 1-px halo baked into its input slab host-side. x is pre-cast to bf16
on the host; all matmuls are bf16 with fp32 PSUM accumulation.

v3 design:
 - kemb grouped conv runs as 4 CONCURRENT column-group matmuls (PE 32-col
   tiling): per group the 3 column taps are folded into a 96-row contraction
   against a DMA-replicated input (3 col-shifted copies); the 3 row taps
   accumulate in PSUM. 24 MMs/tile but only ~6 serial MM-times of wall.
 - xv computed once into a persistent column-padded SBUF slab [C, 66, 258];
   per-tap shifted reads are free strided views.
 - wbar taps split: side-column taps get an ACT psum->sbuf bias-copy then a
   2x bf16 DVE product; center-column taps use a fused scalar_tensor_tensor
   product straight out of PSUM (bias folded).
 - aggregation: part of the product sum runs as identity-matmul PSUM
   accumulation on PE; the rest is a small DVE add tree.
 - half-tile [C,512] PSUM granularity everywhere; poolA 3 banks, poolB 5.
 - SE AllReduce split: partial over tiles 0..11 issued early (hidden under
   tiles 12-15), remainder after tile 15; fused SE epilogue ops.
 - phase 2: most tiles blend on PE via two diagonal matmuls per half plus an
   ACT conversion; every 3rd tile goes down a pure-DVE path instead.
"""

import numpy as np
import ml_dtypes
from contextlib import ExitStack

import concourse.bass as bass
import concourse.tile as tile
from concourse import bacc, mybir
from concourse.bass_utils import run_bass_kernel_spmd

F32 = mybir.dt.float32
BF16 = mybir.dt.bfloat16
AL = mybir.AluOpType
AF = mybir.ActivationFunctionType
BF = ml_dtypes.bfloat16

B, C, H, W = 2, 128, 256, 256
KSZ, SP = 3, 8
NCORES = 8
RQ = H // 4          # 64 rows per core
TR = 4               # output rows per macro-tile
NT = RQ // TR        # 16 macro-tiles per core
NPX = TR * W         # 1024 px per macro-tile
HPX = NPX // 2       # 512 px per half-tile

# ---- tuning knobs ----
PSUM_TAPS = (1, 4, 7)       # taps consumed straight from PSUM via fused STT
PE_ACC_TAPS = (1, 4, 7)     # taps accumulated by PE identity-matmul (+ DVE partial)
SPLIT_T = 12                # tiles in the first (hidden) AllReduce chunk
XV_CONV_DVE = 2             # xv psum->slab conversion: every Nth chunk on DVE
P2_DVE_MOD = 3              # phase2: every Nth tile goes down the pure-DVE path


def _prep_weights(inputs):
    Wk = np.asarray(inputs["Wk"], np.float32)
    We1 = np.asarray(inputs["We1"], np.float32)[:, :, 0, 0]
    We2 = np.asarray(inputs["We2"], np.float32)[:, :, 0, 0]
    be2 = np.asarray(inputs["be2"], np.float32)
    Wv = np.asarray(inputs["Wv"], np.float32)[:, :, 0, 0]
    Ws1 = np.asarray(inputs["Ws1"], np.float32)[:, :, 0, 0]
    bs1 = np.asarray(inputs["bs1"], np.float32)
    Ws2 = np.asarray(inputs["Ws2"], np.float32)[:, :, 0, 0]
    bs2 = np.asarray(inputs["bs2"], np.float32)

    # kemb as 4 column-group matmuls: lhsT[(32b+ci), 3g+a, co] = Wk[32g+co, ci, a, b]
    wk3 = np.zeros((96, 12, 32), np.float32)
    for g in range(4):
        for a in range(3):
            for b in range(3):
                wk3[32 * b:32 * b + 32, 3 * g + a, :] = Wk[32 * g:32 * g + 32, :, a, b].T
    cidx = (np.arange(C) // SP) * 9
    we2 = np.zeros((64, 9, C), np.float32)
    be2k = np.zeros((C, 9), np.float32)
    for t in range(9):
        we2[:, t, :] = We2[cidx + t, :].T
        be2k[:, t] = be2[cidx + t]
    # taps packed pairwise into disjoint 64-row PE groups: even tap at
    # partitions 0-63, odd tap at 64-127 (reads the duplicated w1 half)
    we2p = np.zeros((C, 5, C), np.float32)
    for jj in range(5):
        we2p[0:64, jj, :] = we2[:, 2 * jj, :]
        if jj < 4:
            we2p[64:C, jj, :] = we2[:, 2 * jj + 1, :]
    ws2 = np.zeros((64, 2, C), np.float32)
    ws2[:, 0, :] = Ws2[0::2, :].T
    ws2[:, 1, :] = Ws2[1::2, :].T
    bs2d = (bs2[0::2] - bs2[1::2]).reshape(C, 1)
    w1x2 = np.concatenate([We1[:, :C].T, We1[:, :C].T], axis=1)   # [128,128]
    w1k2 = np.concatenate([We1[:, C:].T, We1[:, C:].T], axis=1)   # [128,128]
    return dict(
        wk3=np.ascontiguousarray(wk3.astype(BF)),
        w1x=np.ascontiguousarray(w1x2.astype(BF)),
        w1k=np.ascontiguousarray(w1k2.astype(BF)),
        we2=np.ascontiguousarray(we2p.astype(BF)),
        be2=np.ascontiguousarray(be2k),
        wv=np.ascontiguousarray(Wv.T.astype(BF)),
        ws1=np.ascontiguousarray((Ws1.T / float(H * W)).astype(np.float32)),
        bs1=bs1.reshape(64, 1),
        ws2=np.ascontiguousarray(ws2),
        bs2d=np.ascontiguousarray(bs2d),
        ident=np.ascontiguousarray(np.eye(C, dtype=np.float32).astype(BF)),
    )


def _build_kernel(nc):
    xs = nc.dram_tensor("xs", [C, RQ + 2, W + 2], BF16, kind="ExternalInput")
    wk3_d = nc.dram_tensor("wk3", [96, 12, 32], BF16, kind="ExternalInput")
    w1x_d = nc.dram_tensor("w1x", [C, C], BF16, kind="ExternalInput")
    w1k_d = nc.dram_tensor("w1k", [C, C], BF16, kind="ExternalInput")
    we2_d = nc.dram_tensor("we2", [C, 5, C], BF16, kind="ExternalInput")
    be2_d = nc.dram_tensor("be2", [C, 9], F32, kind="ExternalInput")
    wv_d = nc.dram_tensor("wv", [C, C], BF16, kind="ExternalInput")
    ws1_d = nc.dram_tensor("ws1", [C, 64], F32, kind="ExternalInput")
    bs1_d = nc.dram_tensor("bs1", [64, 1], F32, kind="ExternalInput")
    ws2_d = nc.dram_tensor("ws2", [64, 2, C], F32, kind="ExternalInput")
    bs2d_d = nc.dram_tensor("bs2d", [C, 1], F32, kind="ExternalInput")
    id_d = nc.dram_tensor("ident", [C, C], BF16, kind="ExternalInput")
    out_d = nc.dram_tensor("out", [C, RQ * W], F32, kind="ExternalOutput")

    cc_inA = nc.dram_tensor("cc_inA", [C, 1], F32, kind="Internal")
    cc_outA = nc.dram_tensor("cc_outA", [C, 1], F32, kind="Internal")
    cc_inB = nc.dram_tensor("cc_inB", [C, 1], F32, kind="Internal")
    cc_outB = nc.dram_tensor("cc_outB", [C, 1], F32, kind="Internal")

    with tile.TileContext(nc) as tc, ExitStack() as ctx:
        singles = ctx.enter_context(tc.tile_pool(name="singles", bufs=1))
        xpool = ctx.enter_context(tc.tile_pool(name="xchunk", bufs=4))
        xrpool = ctx.enter_context(tc.tile_pool(name="xrep", bufs=2))
        w1pool = ctx.enter_context(tc.tile_pool(name="w1p", bufs=3))
        wbpool = ctx.enter_context(tc.tile_pool(name="wbp", bufs=8))
        prodp = ctx.enter_context(tc.tile_pool(name="prodp", bufs=24))
        accp = ctx.enter_context(tc.tile_pool(name="accp", bufs=8))
        outp = ctx.enter_context(tc.tile_pool(name="outp", bufs=4))
        p2pool = ctx.enter_context(tc.tile_pool(name="p2p", bufs=2))
        smallp = ctx.enter_context(tc.tile_pool(name="smallp", bufs=1))
        # PSUM: poolA (xv chunk / kemb / w1 / SE / phase2) 3x[128,512] = 3 banks
        #       poolB (wbar tap halves + agg accumulation)  5x[128,512] = 5 banks
        poolA = ctx.enter_context(tc.tile_pool(name="poolA", bufs=3, space="PSUM"))
        poolB = ctx.enter_context(tc.tile_pool(name="poolB", bufs=5, space="PSUM"))

        def sb(name, shape, dt, dram):
            t_ = singles.tile(shape, dt, tag=name, name=name)
            nc.sync.dma_start(t_, dram.ap())
            return t_

        wk3_sb = sb("wk3", [96, 12, 32], BF16, wk3_d)
        w1x_sb = sb("w1x", [C, C], BF16, w1x_d)
        w1k_sb = sb("w1k", [C, C], BF16, w1k_d)
        we2_sb = sb("we2", [C, 5, C], BF16, we2_d)
        be2_sb = sb("be2", [C, 9], F32, be2_d)
        wv_sb = sb("wv", [C, C], BF16, wv_d)
        ws1_sb = sb("ws1", [C, 64], F32, ws1_d)
        bs1_sb = sb("bs1", [64, 1], F32, bs1_d)
        ws2_sb = sb("ws2", [64, 2, C], F32, ws2_d)
        bs2d_sb = sb("bs2d", [C, 1], F32, bs2d_d)
        id_sb = sb("ident", [C, C], BF16, id_d)

        kemb_slab = singles.tile([C, RQ * W], BF16)
        agg_slab = singles.tile([C, RQ * W], BF16)
        # xv slab rows = xv rows -1..64 (slab row i = xv row i-1); cols 0 and
        # 257 are permanent zero pads giving exact zero-pad tap views.
        xv_slab = singles.tile([C, RQ + 2, W + 2], BF16)
        slots_k = singles.tile([C, 2 * NT], F32)
        slots_a = singles.tile([C, 2 * NT], F32)
        attn_sb = singles.tile([C, 2], F32)
        diag0_sb = singles.tile([C, C], BF16)
        diag1_sb = singles.tile([C, C], BF16)

        nc.gpsimd.memset(xv_slab[:, :, 0:1], 0.0)
        nc.gpsimd.memset(xv_slab[:, :, W + 1:W + 2], 0.0)

        # pre-warm the sigmoid ACT table so the SE tail doesn't pay the
        # ~2.7us table load on the critical path
        warm = smallp.tile([C, 1], F32, tag="warm")
        nc.vector.memset(warm, 0.0)
        nc.scalar.activation(warm, warm, AF.Sigmoid)

        xcs = {}
        xreps = {}

        def dma_xc(t):
            xc = xpool.tile([C, TR + 2, W + 2], BF16, tag="xc")
            nc.sync.dma_start(xc, xs.ap()[:, TR * t:TR * t + TR + 2, :])
            xcs[t] = xc
            # kemb input replicas: per group g, 3 col-shifted copies of its
            # 32 channels stacked on partitions (32b+ci)
            reps = []
            for g in range(4):
                xr = xrpool.tile([96, TR + 2, W], BF16, tag=f"xr{g}",
                                 name=f"xr{g}")
                for b in range(3):
                    nc.sync.dma_start(
                        xr[32 * b:32 * b + 32, :, :],
                        xs.ap()[32 * g:32 * g + 32,
                                TR * t:TR * t + TR + 2, b:b + W])
                reps.append(xr)
            xreps[t] = reps

        def xv_chunk(c, xc, rlo, rhi):
            # xv rows [rlo, rhi) of xc-local rows -> slab rows rlo+1..
            nrows = rhi - rlo
            for h0 in range(0, nrows, 2):
                pxv = poolA.tile([C, HPX], F32, tag="pA", name="pxv")
                nc.tensor.matmul(pxv, lhsT=wv_sb,
                                 rhs=xc[:, h0 + rlo + 1:h0 + rlo + 3, 1:1 + W],
                                 start=True, stop=True)
                pv = pxv.rearrange("p (r w) -> p r w", w=W)
                # slab row = xv row + 1; xv row 4c-1+h0 -> slab row 4c+h0
                r0 = 4 * c + h0
                dst = xv_slab[:, r0:r0 + 2, 1:1 + W]
                if XV_CONV_DVE and ((c + h0) % XV_CONV_DVE == 0):
                    nc.vector.tensor_scalar(dst, pv, 0.0, None, AL.add)
                else:
                    nc.scalar.activation(dst, pv, AF.Copy)

        # prologue: first input chunk + first xv chunk (xv rows -1..2 live in
        # xc(0) local rows 0..3)
        dma_xc(0)
        xv_chunk(0, xcs[0], -1, 3)

        def emit_se(tag, lo, hi, cc_in, cc_out):
            rk = smallp.tile([C, 1], F32, tag=f"rk{tag}", name=f"rk{tag}")
            ra = smallp.tile([C, 1], F32, tag=f"ra{tag}", name=f"ra{tag}")
            nc.vector.tensor_reduce(rk, slots_k[:, 2 * lo:2 * hi],
                                    mybir.AxisListType.X, AL.add)
            nc.vector.tensor_reduce(ra, slots_a[:, 2 * lo:2 * hi],
                                    mybir.AxisListType.X, AL.add)
            gap = smallp.tile([C, 1], F32, tag=f"gap{tag}", name=f"gap{tag}")
            nc.vector.tensor_tensor(gap, rk, ra, AL.add)
            nc.gpsimd.dma_start(cc_in.ap(), gap)
            nc.gpsimd.collective_compute(
                "AllReduce", AL.add,
                replica_groups=[[0, 1, 2, 3], [4, 5, 6, 7]],
                ins=[cc_in.ap().opt()],
                outs=[cc_out.ap().opt()],
            )
            g2 = smallp.tile([C, 1], F32, tag=f"g2{tag}", name=f"g2{tag}")
            nc.gpsimd.dma_start(g2, cc_out.ap())
            return g2

        g2A = None

        # ---------------- phase 1 ----------------
        for t in range(NT):
            # prefetch next input tile + compute xv chunk t+1 (tile t's
            # products need xv slab rows up to 4t+5 = chunk t+1)
            if t + 1 < NT:
                dma_xc(t + 1)
                xv_chunk(t + 1, xcs[t + 1], -1, 3)
            else:
                # epilogue chunk: xv rows 63..64 -> slab rows 64..65
                xv_chunk(t + 1, xcs[NT - 1], 3, 5)

            xc = xcs[t]
            reps = xreps[t]

            # kemb: 4 concurrent column-group matmuls, 3 row-taps accumulate
            kvh = [None, None]
            for h in range(2):
                pk = poolA.tile([C, HPX], F32, tag="pA", name="pk")
                for a in range(3):
                    for g in range(4):
                        nc.tensor.matmul(
                            pk[32 * g:32 * g + 32, :],
                            lhsT=wk3_sb[0:96, 3 * g + a, :],
                            rhs=reps[g][:, 2 * h + a:2 * h + a + 2, :],
                            start=(a == 0), stop=(a == 2),
                            tile_position=(0, 32 * g))
                kv = kemb_slab[:, t * NPX + h * HPX:t * NPX + (h + 1) * HPX]
                nc.scalar.activation(kv, pk, AF.Relu,
                                     accum_out=slots_k[:, 2 * t + h:2 * t + h + 1])
                kvh[h] = kv

            # w1 = relu(We1 @ [x; kemb]), duplicated into both 64-row halves
            w1b = w1pool.tile([C, NPX], BF16, tag="w1")
            for h in range(2):
                pw = poolA.tile([C, HPX], F32, tag="pA", name="pw")
                nc.tensor.matmul(pw, lhsT=w1x_sb,
                                 rhs=xc[:, 1 + 2 * h:3 + 2 * h, 1:1 + W],
                                 start=True, stop=False)
                nc.tensor.matmul(pw, lhsT=w1k_sb, rhs=kvh[h],
                                 start=False, stop=True)
                nc.scalar.activation(w1b[:, h * HPX:(h + 1) * HPX], pw, AF.Relu)

            # wbar taps (paired into disjoint 64-row PE groups) + products
            def xv_view(tap, h):
                a, b = divmod(tap, 3)
                r0 = 4 * t + 2 * h + a
                return xv_slab[:, r0:r0 + 2, b:b + W]

            prods = [[None, None] for _ in range(9)]
            for jj in range(5):
                taps = [2 * jj] + ([2 * jj + 1] if jj < 4 else [])
                for h in range(2):
                    cs = slice(h * HPX, (h + 1) * HPX)
                    pbs = {}
                    for ti, tap in enumerate(taps):
                        pbs[tap] = poolB.tile([C, HPX], F32, tag="pB",
                                              name=f"pb{ti}")
                        lo = 64 * ti
                        nc.tensor.matmul(
                            pbs[tap],
                            lhsT=we2_sb[lo:lo + 64, jj, :],
                            rhs=w1b[lo:lo + 64, cs],
                            start=True, stop=True)
                    for tap in taps:
                        p = prodp.tile([C, HPX], BF16, tag="prod", name="prod")
                        if tap in PSUM_TAPS:
                            nc.vector.scalar_tensor_tensor(
                                p, pbs[tap], be2_sb[:, tap:tap + 1],
                                xv_view(tap, h), AL.add, AL.mult)
                        else:
                            wb = wbpool.tile([C, HPX], BF16, tag="wb",
                                             name="wb")
                            nc.scalar.activation(wb, pbs[tap], AF.Identity,
                                                 bias=be2_sb[:, tap:tap + 1])
                            nc.vector.tensor_tensor(p, wb, xv_view(tap, h),
                                                    AL.mult)
                        prods[tap][h] = p

            # aggregation per half: DVE partial tree + PE identity-matmul
            dve_taps = [k for k in range(9) if k not in PE_ACC_TAPS]
            for h in range(2):
                sums = [prods[k][h] for k in dve_taps]
                while len(sums) > 1:
                    s = accp.tile([C, HPX], BF16, tag="acc", name="acc")
                    nc.vector.tensor_tensor(s, sums[0], sums[1], AL.add)
                    sums = sums[2:] + [s]
                pe_rhs = [prods[k][h] for k in PE_ACC_TAPS] + sums
                pagg = poolB.tile([C, HPX], F32, tag="pB", name="pagg")
                n = len(pe_rhs)
                for i, r in enumerate(pe_rhs):
                    nc.tensor.matmul(pagg, lhsT=id_sb, rhs=r,
                                     start=(i == 0), stop=(i == n - 1))
                av = agg_slab[:, t * NPX + h * HPX:t * NPX + (h + 1) * HPX]
                nc.scalar.activation(av, pagg, AF.Relu,
                                     accum_out=slots_a[:, 2 * t + h:2 * t + h + 1])

            if t == SPLIT_T - 1:
                g2A = emit_se("A", 0, SPLIT_T, cc_inA, cc_outA)

        # ---------------- SE attention tail ----------------
        g2B = emit_se("B", SPLIT_T, NT, cc_inB, cc_outB)
        gap2 = smallp.tile([C, 1], F32, tag="gapT")
        nc.vector.tensor_tensor(gap2, g2A, g2B, AL.add)

        ph = poolA.tile([64, 1], F32, tag="pA", name="ph")
        nc.tensor.matmul(ph, lhsT=ws1_sb, rhs=gap2, start=True, stop=True)
        hso = smallp.tile([64, 1], F32, tag="h")
        nc.scalar.activation(hso, ph, AF.Relu, bias=bs1_sb[:, 0:1])
        pa = poolA.tile([C, 2], F32, tag="pA", name="pa")
        nc.tensor.matmul(pa[:, 0:1], lhsT=ws2_sb[:, 0, :], rhs=hso,
                         start=True, stop=True)
        nc.tensor.matmul(pa[:, 1:2], lhsT=ws2_sb[:, 1, :], rhs=hso,
                         start=True, stop=True)
        # dse = (a0 + (bs2_0 - bs2_1)) - a1  (one psum->sbuf hop, then fused)
        a01 = smallp.tile([C, 2], F32, tag="a01")
        nc.scalar.activation(a01, pa, AF.Copy)
        dse = smallp.tile([C, 1], F32, tag="dse")
        nc.vector.scalar_tensor_tensor(dse, a01[:, 0:1], bs2d_sb[:, 0:1],
                                       a01[:, 1:2], AL.add, AL.subtract)
        nc.scalar.activation(attn_sb[:, 0:1], dse, AF.Sigmoid)
        nc.scalar.activation(attn_sb[:, 1:2], dse, AF.Sigmoid, scale=-1.0)
        nc.vector.tensor_scalar(diag0_sb, id_sb, attn_sb[:, 0:1], None, AL.mult)
        nc.vector.tensor_scalar(diag1_sb, id_sb, attn_sb[:, 1:2], None, AL.mult)

        # ---------------- phase 2: blend + store ----------------
        for t in range(NT):
            kv = kemb_slab[:, t * NPX:(t + 1) * NPX]
            av = agg_slab[:, t * NPX:(t + 1) * NPX]
            outf = outp.tile([C, NPX], F32, tag="outf")
            if P2_DVE_MOD and (t % P2_DVE_MOD == P2_DVE_MOD - 1):
                t1 = p2pool.tile([C, NPX], BF16, tag="t1")
                nc.vector.tensor_scalar(t1, kv, attn_sb[:, 1:2], None, AL.mult)
                nc.vector.scalar_tensor_tensor(outf, av, attn_sb[:, 0:1], t1,
                                               AL.mult, AL.add)
            else:
                for h in range(2):
                    cs = slice(h * HPX, (h + 1) * HPX)
                    p2 = poolA.tile([C, HPX], F32, tag="pA", name="p2")
                    nc.tensor.matmul(p2, lhsT=diag0_sb, rhs=av[:, cs],
                                     start=True, stop=False)
                    nc.tensor.matmul(p2, lhsT=diag1_sb, rhs=kv[:, cs],
                                     start=False, stop=True)
                    nc.scalar.activation(outf[:, cs], p2, AF.Copy)
            nc.sync.dma_start(out_d.ap()[:, t * NPX:(t + 1) * NPX], outf)

    return nc


_CACHE = {}


def _get_nc():
    if "nc" not in _CACHE:
        nc = bacc.Bacc("TRN2", target_bir_lowering=False, debug=False,
                       num_devices=NCORES)
        _build_kernel(nc)
        nc.compile()
        _CACHE["nc"] = nc
    return _CACHE["nc"]


def make_in_maps(inputs):
    x = np.asarray(inputs["x"], np.float32)
    wts = _prep_weights(inputs)
    xp = np.pad(x, ((0, 0), (0, 0), (1, 1), (1, 1))).astype(BF)
    in_maps = []
    for core in range(NCORES):
        bb, q = divmod(core, 4)
        slab = np.ascontiguousarray(xp[bb, :, RQ * q:RQ * q + RQ + 2, :])
        m = {"xs": slab}
        m.update(wts)
        in_maps.append(m)
    return in_maps


def kernel(**inputs):
    in_maps = make_in_maps(inputs)
    nc = _get_nc()
    res = run_bass_kernel_spmd(nc, in_maps, core_ids=list(range(NCORES)))
    out = np.empty((B, C, H, W), np.float32)
    for core in range(NCORES):
        bb, q = divmod(core, 4)
        out[bb, :, RQ * q:RQ * q + RQ, :] = \
            res.results[core]["out"].reshape(C, RQ, W)
    return out


# revision 9
# speedup vs baseline: 1.1801x; 1.1801x over previous
"""Trainium2 Bass kernel for nn_CotLayer (CoT attention layer).

Computation (see reference):
  kemb = relu(grouped_conv3x3(x, Wk, groups=4))
  w1   = relu(We1 @ [x; kemb])            (1x1)
  wbar_k = We2_k @ w1 + be2_k             (per-pixel 3x3 kernel, 8-fold
                                           group replication folded into We2_k)
  xv   = Wv @ x                           (1x1)
  agg  = relu(sum_k shift_k(xv) * wbar_k)
  gap  = mean_{H,W}(agg + kemb)           (AllReduce across 4-core groups)
  attn = sigmoid pair of SE-MLP(gap)
  out  = agg*attn0 + kemb*attn1

Sharding: 8 cores = (batch b) x (H-quarter q); each core computes 64 output
rows; 1-px halo baked into its input slab host-side. x is pre-cast to bf16
on the host; all matmuls are bf16 with fp32 PSUM accumulation.

v3 design:
 - kemb grouped conv runs as 4 CONCURRENT column-group matmuls (PE 32-col
   tiling): per group the 3 column taps are folded into a 96-row contraction
   against a DMA-replicated input (3 col-shifted copies); the 3 row taps
   accumulate in PSUM. 24 MMs/tile but only ~6 serial MM-times of wall.
 - xv computed once into a persistent column-padded SBUF slab [C, 66, 258];
   per-tap shifted reads are free strided views.
 - wbar taps split: side-column taps get an ACT psum->sbuf bias-copy then a
   2x bf16 DVE product; center-column taps use a fused scalar_tensor_tensor
   product straight out of PSUM (bias folded).
 - aggregation: part of the product sum runs as identity-matmul PSUM
   accumulation on PE; the rest is a small DVE add tree.
 - half-tile [C,512] PSUM granularity everywhere; poolA 3 banks, poolB 5.
 - SE AllReduce split: partial over tiles 0..11 issued early (hidden under
   tiles 12-15), remainder after tile 15; fused SE epilogue ops.
 - phase 2: most tiles blend on PE via two diagonal matmuls per half plus an
   ACT conversion; every 3rd tile goes down a pure-DVE path instead.
"""

import numpy as np
import ml_dtypes
from contextlib import ExitStack

import concourse.bass as bass
import concourse.tile as tile
from concourse import bacc, mybir
from concourse.bass_utils import run_bass_kernel_spmd

F32 = mybir.dt.float32
BF16 = mybir.dt.bfloat16
AL = mybir.AluOpType
AF = mybir.ActivationFunctionType
BF = ml_dtypes.bfloat16

B, C, H, W = 2, 128, 256, 256
KSZ, SP = 3, 8
NCORES = 8
RQ = H // 4          # 64 rows per core
TR = 4               # output rows per macro-tile
NT = RQ // TR        # 16 macro-tiles per core
NPX = TR * W         # 1024 px per macro-tile
HPX = NPX // 2       # 512 px per half-tile

# ---- tuning knobs ----
PSUM_TAPS = (1, 4, 7)       # taps consumed straight from PSUM via fused STT
PE_ACC_TAPS = (1, 4, 7)     # taps accumulated by PE identity-matmul (+ DVE partial)
SPLIT_T = 12                # tiles in the first (hidden) AllReduce chunk
XV_CONV_DVE = 2             # xv psum->slab conversion: every Nth chunk on DVE
P2_DVE_MOD = 3              # phase2: every Nth tile goes down the pure-DVE path


def _prep_weights(inputs):
    Wk = np.asarray(inputs["Wk"], np.float32)
    We1 = np.asarray(inputs["We1"], np.float32)[:, :, 0, 0]
    We2 = np.asarray(inputs["We2"], np.float32)[:, :, 0, 0]
    be2 = np.asarray(inputs["be2"], np.float32)
    Wv = np.asarray(inputs["Wv"], np.float32)[:, :, 0, 0]
    Ws1 = np.asarray(inputs["Ws1"], np.float32)[:, :, 0, 0]
    bs1 = np.asarray(inputs["bs1"], np.float32)
    Ws2 = np.asarray(inputs["Ws2"], np.float32)[:, :, 0, 0]
    bs2 = np.asarray(inputs["bs2"], np.float32)

    # kemb as 4 column-group matmuls: lhsT[(32b+ci), 3g+a, co] = Wk[32g+co, ci, a, b]
    wk3 = np.zeros((96, 12, 32), np.float32)
    for g in range(4):
        for a in range(3):
            for b in range(3):
                wk3[32 * b:32 * b + 32, 3 * g + a, :] = Wk[32 * g:32 * g + 32, :, a, b].T
    cidx = (np.arange(C) // SP) * 9
    we2 = np.zeros((64, 9, C), np.float32)
    be2k = np.zeros((C, 9), np.float32)
    for t in range(9):
        we2[:, t, :] = We2[cidx + t, :].T
        be2k[:, t] = be2[cidx + t]
    # taps packed pairwise into disjoint 64-row PE groups: even tap at
    # partitions 0-63, odd tap at 64-127 (reads the duplicated w1 half)
    we2p = np.zeros((C, 5, C), np.float32)
    for jj in range(5):
        we2p[0:64, jj, :] = we2[:, 2 * jj, :]
        if jj < 4:
            we2p[64:C, jj, :] = we2[:, 2 * jj + 1, :]
    ws2 = np.zeros((64, 2, C), np.float32)
    ws2[:, 0, :] = Ws2[0::2, :].T
    ws2[:, 1, :] = Ws2[1::2, :].T
    bs2d = (bs2[0::2] - bs2[1::2]).reshape(C, 1)
    w1x2 = np.concatenate([We1[:, :C].T, We1[:, :C].T], axis=1)   # [128,128]
    w1k2 = np.concatenate([We1[:, C:].T, We1[:, C:].T], axis=1)   # [128,128]
    return dict(
        wk3=np.ascontiguousarray(wk3.astype(BF)),
        w1x=np.ascontiguousarray(w1x2.astype(BF)),
        w1k=np.ascontiguousarray(w1k2.astype(BF)),
        we2=np.ascontiguousarray(we2p.astype(BF)),
        be2=np.ascontiguousarray(be2k),
        wv=np.ascontiguousarray(Wv.T.astype(BF)),
        ws1=np.ascontiguousarray((Ws1.T / float(H * W)).astype(np.float32)),
        bs1=bs1.reshape(64, 1),
        ws2=np.ascontiguousarray(ws2),
        bs2d=np.ascontiguousarray(bs2d),
        ident=np.ascontiguousarray(np.eye(C, dtype=np.float32).astype(BF)),
    )


def _build_kernel(nc):
    xs = nc.dram_tensor("xs", [C, RQ + 2, W + 2], BF16, kind="ExternalInput")
    xsr = nc.dram_tensor("xsr", [96, 4, RQ + 2, W], BF16, kind="ExternalInput")
    wk3_d = nc.dram_tensor("wk3", [96, 12, 32], BF16, kind="ExternalInput")
    w1x_d = nc.dram_tensor("w1x", [C, C], BF16, kind="ExternalInput")
    w1k_d = nc.dram_tensor("w1k", [C, C], BF16, kind="ExternalInput")
    we2_d = nc.dram_tensor("we2", [C, 5, C], BF16, kind="ExternalInput")
    be2_d = nc.dram_tensor("be2", [C, 9], F32, kind="ExternalInput")
    wv_d = nc.dram_tensor("wv", [C, C], BF16, kind="ExternalInput")
    ws1_d = nc.dram_tensor("ws1", [C, 64], F32, kind="ExternalInput")
    bs1_d = nc.dram_tensor("bs1", [64, 1], F32, kind="ExternalInput")
    ws2_d = nc.dram_tensor("ws2", [64, 2, C], F32, kind="ExternalInput")
    bs2d_d = nc.dram_tensor("bs2d", [C, 1], F32, kind="ExternalInput")
    id_d = nc.dram_tensor("ident", [C, C], BF16, kind="ExternalInput")
    out_d = nc.dram_tensor("out", [C, RQ * W], F32, kind="ExternalOutput")

    cc_inA = nc.dram_tensor("cc_inA", [C, 1], F32, kind="Internal")
    cc_outA = nc.dram_tensor("cc_outA", [C, 1], F32, kind="Internal")
    cc_inB = nc.dram_tensor("cc_inB", [C, 1], F32, kind="Internal")
    cc_outB = nc.dram_tensor("cc_outB", [C, 1], F32, kind="Internal")

    with tile.TileContext(nc) as tc, ExitStack() as ctx:
        singles = ctx.enter_context(tc.tile_pool(name="singles", bufs=1))
        xpool = ctx.enter_context(tc.tile_pool(name="xchunk", bufs=4))
        xrpool = ctx.enter_context(tc.tile_pool(name="xrep", bufs=2))
        w1pool = ctx.enter_context(tc.tile_pool(name="w1p", bufs=3))
        wbpool = ctx.enter_context(tc.tile_pool(name="wbp", bufs=8))
        prodp = ctx.enter_context(tc.tile_pool(name="prodp", bufs=24))
        accp = ctx.enter_context(tc.tile_pool(name="accp", bufs=8))
        outp = ctx.enter_context(tc.tile_pool(name="outp", bufs=4))
        p2pool = ctx.enter_context(tc.tile_pool(name="p2p", bufs=2))
        smallp = ctx.enter_context(tc.tile_pool(name="smallp", bufs=1))
        # PSUM: poolA (xv chunk / kemb / w1 / SE / phase2) 3x[128,512] = 3 banks
        #       poolB (wbar tap halves + agg accumulation)  5x[128,512] = 5 banks
        poolA = ctx.enter_context(tc.tile_pool(name="poolA", bufs=3, space="PSUM"))
        poolB = ctx.enter_context(tc.tile_pool(name="poolB", bufs=5, space="PSUM"))

        def sb(name, shape, dt, dram):
            t_ = singles.tile(shape, dt, tag=name, name=name)
            nc.sync.dma_start(t_, dram.ap())
            return t_

        wk3_sb = sb("wk3", [96, 12, 32], BF16, wk3_d)
        w1x_sb = sb("w1x", [C, C], BF16, w1x_d)
        w1k_sb = sb("w1k", [C, C], BF16, w1k_d)
        we2_sb = sb("we2", [C, 5, C], BF16, we2_d)
        be2_sb = sb("be2", [C, 9], F32, be2_d)
        wv_sb = sb("wv", [C, C], BF16, wv_d)
        ws1_sb = sb("ws1", [C, 64], F32, ws1_d)
        bs1_sb = sb("bs1", [64, 1], F32, bs1_d)
        ws2_sb = sb("ws2", [64, 2, C], F32, ws2_d)
        bs2d_sb = sb("bs2d", [C, 1], F32, bs2d_d)
        id_sb = sb("ident", [C, C], BF16, id_d)

        kemb_slab = singles.tile([C, RQ * W], BF16)
        agg_slab = singles.tile([C, RQ * W], BF16)
        # xv slab rows = xv rows -1..64 (slab row i = xv row i-1); cols 0 and
        # 257 are permanent zero pads giving exact zero-pad tap views.
        xv_slab = singles.tile([C, RQ + 2, W + 2], BF16)
        slots_k = singles.tile([C, 2 * NT], F32)
        slots_a = singles.tile([C, 2 * NT], F32)
        attn_sb = singles.tile([C, 2], F32)
        diag0_sb = singles.tile([C, C], BF16)
        diag1_sb = singles.tile([C, C], BF16)

        nc.gpsimd.memset(xv_slab[:, :, 0:1], 0.0)
        nc.gpsimd.memset(xv_slab[:, :, W + 1:W + 2], 0.0)

        # pre-warm the sigmoid ACT table so the SE tail doesn't pay the
        # ~2.7us table load on the critical path
        warm = smallp.tile([C, 1], F32, tag="warm")
        nc.vector.memset(warm, 0.0)
        nc.scalar.activation(warm, warm, AF.Sigmoid)

        xcs = {}
        xreps = {}

        def dma_xc(t):
            xc = xpool.tile([C, TR + 2, W + 2], BF16, tag="xc")
            nc.sync.dma_start(xc, xs.ap()[:, TR * t:TR * t + TR + 2, :])
            xcs[t] = xc
            # kemb input replicas (host-prepared contiguous layout): one DMA
            # with 3KB-per-partition runs brings all 4 groups' shifted copies
            xr = xrpool.tile([96, 4, TR + 2, W], BF16, tag="xr", name="xr")
            nc.sync.dma_start(xr, xsr.ap()[:, :, TR * t:TR * t + TR + 2, :])
            xreps[t] = xr

        def xv_chunk(c, xc, rlo, rhi):
            # xv rows [rlo, rhi) of xc-local rows -> slab rows rlo+1..
            nrows = rhi - rlo
            for h0 in range(0, nrows, 2):
                pxv = poolA.tile([C, HPX], F32, tag="pA", name="pxv")
                nc.tensor.matmul(pxv, lhsT=wv_sb,
                                 rhs=xc[:, h0 + rlo + 1:h0 + rlo + 3, 1:1 + W],
                                 start=True, stop=True)
                pv = pxv.rearrange("p (r w) -> p r w", w=W)
                # slab row = xv row + 1; xv row 4c-1+h0 -> slab row 4c+h0
                r0 = 4 * c + h0
                dst = xv_slab[:, r0:r0 + 2, 1:1 + W]
                if XV_CONV_DVE and ((c + h0) % XV_CONV_DVE == 0):
                    nc.vector.tensor_scalar(dst, pv, 0.0, None, AL.add)
                else:
                    nc.scalar.activation(dst, pv, AF.Copy)

        # prologue: first input chunk + first xv chunk (xv rows -1..2 live in
        # xc(0) local rows 0..3)
        dma_xc(0)
        xv_chunk(0, xcs[0], -1, 3)

        def emit_se(tag, lo, hi, cc_in, cc_out):
            rk = smallp.tile([C, 1], F32, tag=f"rk{tag}", name=f"rk{tag}")
            ra = smallp.tile([C, 1], F32, tag=f"ra{tag}", name=f"ra{tag}")
            nc.vector.tensor_reduce(rk, slots_k[:, 2 * lo:2 * hi],
                                    mybir.AxisListType.X, AL.add)
            nc.vector.tensor_reduce(ra, slots_a[:, 2 * lo:2 * hi],
                                    mybir.AxisListType.X, AL.add)
            gap = smallp.tile([C, 1], F32, tag=f"gap{tag}", name=f"gap{tag}")
            nc.vector.tensor_tensor(gap, rk, ra, AL.add)
            nc.gpsimd.dma_start(cc_in.ap(), gap)
            nc.gpsimd.collective_compute(
                "AllReduce", AL.add,
                replica_groups=[[0, 1, 2, 3], [4, 5, 6, 7]],
                ins=[cc_in.ap().opt()],
                outs=[cc_out.ap().opt()],
            )
            g2 = smallp.tile([C, 1], F32, tag=f"g2{tag}", name=f"g2{tag}")
            nc.gpsimd.dma_start(g2, cc_out.ap())
            return g2

        g2A = None

        # ---------------- phase 1 ----------------
        for t in range(NT):
            # prefetch next input tile + compute xv chunk t+1 (tile t's
            # products need xv slab rows up to 4t+5 = chunk t+1)
            if t + 1 < NT:
                dma_xc(t + 1)
                xv_chunk(t + 1, xcs[t + 1], -1, 3)
            else:
                # epilogue chunk: xv rows 63..64 -> slab rows 64..65
                xv_chunk(t + 1, xcs[NT - 1], 3, 5)

            xc = xcs[t]
            xr = xreps[t]

            # kemb: 4 concurrent column-group matmuls, 3 row-taps accumulate
            kvh = [None, None]
            for h in range(2):
                pk = poolA.tile([C, HPX], F32, tag="pA", name="pk")
                for a in range(3):
                    for g in range(4):
                        nc.tensor.matmul(
                            pk[32 * g:32 * g + 32, :],
                            lhsT=wk3_sb[0:96, 3 * g + a, :],
                            rhs=xr[:, g, 2 * h + a:2 * h + a + 2, :],
                            start=(a == 0), stop=(a == 2),
                            tile_position=(0, 32 * g))
                kv = kemb_slab[:, t * NPX + h * HPX:t * NPX + (h + 1) * HPX]
                nc.scalar.activation(kv, pk, AF.Relu,
                                     accum_out=slots_k[:, 2 * t + h:2 * t + h + 1])
                kvh[h] = kv

            # w1 = relu(We1 @ [x; kemb]), duplicated into both 64-row halves
            w1b = w1pool.tile([C, NPX], BF16, tag="w1")
            for h in range(2):
                pw = poolA.tile([C, HPX], F32, tag="pA", name="pw")
                nc.tensor.matmul(pw, lhsT=w1x_sb,
                                 rhs=xc[:, 1 + 2 * h:3 + 2 * h, 1:1 + W],
                                 start=True, stop=False)
                nc.tensor.matmul(pw, lhsT=w1k_sb, rhs=kvh[h],
                                 start=False, stop=True)
                nc.scalar.activation(w1b[:, h * HPX:(h + 1) * HPX], pw, AF.Relu)

            # wbar taps (paired into disjoint 64-row PE groups) + products
            def xv_view(tap, h):
                a, b = divmod(tap, 3)
                r0 = 4 * t + 2 * h + a
                return xv_slab[:, r0:r0 + 2, b:b + W]

            prods = [[None, None] for _ in range(9)]
            for jj in range(5):
                taps = [2 * jj] + ([2 * jj + 1] if jj < 4 else [])
                for h in range(2):
                    cs = slice(h * HPX, (h + 1) * HPX)
                    pbs = {}
                    for ti, tap in enumerate(taps):
                        pbs[tap] = poolB.tile([C, HPX], F32, tag="pB",
                                              name=f"pb{ti}")
                        lo = 64 * ti
                        nc.tensor.matmul(
                            pbs[tap],
                            lhsT=we2_sb[lo:lo + 64, jj, :],
                            rhs=w1b[lo:lo + 64, cs],
                            start=True, stop=True)
                    for tap in taps:
                        p = prodp.tile([C, HPX], BF16, tag="prod", name="prod")
                        if tap in PSUM_TAPS:
                            nc.vector.scalar_tensor_tensor(
                                p, pbs[tap], be2_sb[:, tap:tap + 1],
                                xv_view(tap, h), AL.add, AL.mult)
                        else:
                            wb = wbpool.tile([C, HPX], BF16, tag="wb",
                                             name="wb")
                            nc.scalar.activation(wb, pbs[tap], AF.Identity,
                                                 bias=be2_sb[:, tap:tap + 1])
                            nc.vector.tensor_tensor(p, wb, xv_view(tap, h),
                                                    AL.mult)
                        prods[tap][h] = p

            # aggregation per half: DVE partial tree + PE identity-matmul
            dve_taps = [k for k in range(9) if k not in PE_ACC_TAPS]
            for h in range(2):
                sums = [prods[k][h] for k in dve_taps]
                while len(sums) > 1:
                    s = accp.tile([C, HPX], BF16, tag="acc", name="acc")
                    nc.vector.tensor_tensor(s, sums[0], sums[1], AL.add)
                    sums = sums[2:] + [s]
                pe_rhs = [prods[k][h] for k in PE_ACC_TAPS] + sums
                pagg = poolB.tile([C, HPX], F32, tag="pB", name="pagg")
                n = len(pe_rhs)
                for i, r in enumerate(pe_rhs):
                    nc.tensor.matmul(pagg, lhsT=id_sb, rhs=r,
                                     start=(i == 0), stop=(i == n - 1))
                av = agg_slab[:, t * NPX + h * HPX:t * NPX + (h + 1) * HPX]
                nc.scalar.activation(av, pagg, AF.Relu,
                                     accum_out=slots_a[:, 2 * t + h:2 * t + h + 1])

            if t == SPLIT_T - 1:
                g2A = emit_se("A", 0, SPLIT_T, cc_inA, cc_outA)

        # ---------------- SE attention tail ----------------
        g2B = emit_se("B", SPLIT_T, NT, cc_inB, cc_outB)
        gap2 = smallp.tile([C, 1], F32, tag="gapT")
        nc.vector.tensor_tensor(gap2, g2A, g2B, AL.add)

        ph = poolA.tile([64, 1], F32, tag="pA", name="ph")
        nc.tensor.matmul(ph, lhsT=ws1_sb, rhs=gap2, start=True, stop=True)
        hso = smallp.tile([64, 1], F32, tag="h")
        nc.scalar.activation(hso, ph, AF.Relu, bias=bs1_sb[:, 0:1])
        pa = poolA.tile([C, 2], F32, tag="pA", name="pa")
        nc.tensor.matmul(pa[:, 0:1], lhsT=ws2_sb[:, 0, :], rhs=hso,
                         start=True, stop=True)
        nc.tensor.matmul(pa[:, 1:2], lhsT=ws2_sb[:, 1, :], rhs=hso,
                         start=True, stop=True)
        # dse = (a0 + (bs2_0 - bs2_1)) - a1  (one psum->sbuf hop, then fused)
        a01 = smallp.tile([C, 2], F32, tag="a01")
        nc.scalar.activation(a01, pa, AF.Copy)
        dse = smallp.tile([C, 1], F32, tag="dse")
        nc.vector.scalar_tensor_tensor(dse, a01[:, 0:1], bs2d_sb[:, 0:1],
                                       a01[:, 1:2], AL.add, AL.subtract)
        nc.scalar.activation(attn_sb[:, 0:1], dse, AF.Sigmoid)
        nc.scalar.activation(attn_sb[:, 1:2], dse, AF.Sigmoid, scale=-1.0)
        nc.vector.tensor_scalar(diag0_sb, id_sb, attn_sb[:, 0:1], None, AL.mult)
        nc.vector.tensor_scalar(diag1_sb, id_sb, attn_sb[:, 1:2], None, AL.mult)

        # ---------------- phase 2: blend + store ----------------
        for t in range(NT):
            kv = kemb_slab[:, t * NPX:(t + 1) * NPX]
            av = agg_slab[:, t * NPX:(t + 1) * NPX]
            outf = outp.tile([C, NPX], F32, tag="outf")
            if P2_DVE_MOD and (t % P2_DVE_MOD == P2_DVE_MOD - 1):
                t1 = p2pool.tile([C, NPX], BF16, tag="t1")
                nc.vector.tensor_scalar(t1, kv, attn_sb[:, 1:2], None, AL.mult)
                nc.vector.scalar_tensor_tensor(outf, av, attn_sb[:, 0:1], t1,
                                               AL.mult, AL.add)
            else:
                for h in range(2):
                    cs = slice(h * HPX, (h + 1) * HPX)
                    p2 = poolA.tile([C, HPX], F32, tag="pA", name="p2")
                    nc.tensor.matmul(p2, lhsT=diag0_sb, rhs=av[:, cs],
                                     start=True, stop=False)
                    nc.tensor.matmul(p2, lhsT=diag1_sb, rhs=kv[:, cs],
                                     start=False, stop=True)
                    nc.scalar.activation(outf[:, cs], p2, AF.Copy)
            nc.sync.dma_start(out_d.ap()[:, t * NPX:(t + 1) * NPX], outf)

    return nc


_CACHE = {}


def _get_nc():
    if "nc" not in _CACHE:
        nc = bacc.Bacc("TRN2", target_bir_lowering=False, debug=False,
                       num_devices=NCORES)
        _build_kernel(nc)
        nc.compile()
        _CACHE["nc"] = nc
    return _CACHE["nc"]


def make_in_maps(inputs):
    x = np.asarray(inputs["x"], np.float32)
    wts = _prep_weights(inputs)
    xp = np.pad(x, ((0, 0), (0, 0), (1, 1), (1, 1))).astype(BF)
    in_maps = []
    for core in range(NCORES):
        bb, q = divmod(core, 4)
        slab = np.ascontiguousarray(xp[bb, :, RQ * q:RQ * q + RQ + 2, :])
        rep = np.zeros((96, 4, RQ + 2, W), slab.dtype)
        for g in range(4):
            for b3 in range(3):
                rep[32 * b3:32 * b3 + 32, g] = \
                    slab[32 * g:32 * g + 32, :, b3:b3 + W]
        m = {"xs": slab, "xsr": np.ascontiguousarray(rep)}
        m.update(wts)
        in_maps.append(m)
    return in_maps


def kernel(**inputs):
    in_maps = make_in_maps(inputs)
    nc = _get_nc()
    res = run_bass_kernel_spmd(nc, in_maps, core_ids=list(range(NCORES)))
    out = np.empty((B, C, H, W), np.float32)
    for core in range(NCORES):
        bb, q = divmod(core, 4)
        out[bb, :, RQ * q:RQ * q + RQ, :] = \
            res.results[core]["out"].reshape(C, RQ, W)
    return out
